# revision 29
# baseline (speedup 1.0000x reference)
"""DenseCaptioner LSTM-gate kernel for 8 Trainium2 NeuronCores.

Role-split sharding (halves per-core HBM traffic vs. gate+batch-half
data parallelism):
  cores 0-3  run program VIS: visual + recurrent paths for gate g = core,
             full batch (two 128-row m-tiles)  -> partial logits [256,1024]
  cores 4-7  run program INP: input path for gate g = core-4, full batch
             -> partial logits [256,1024]
Host: logits[g] = vis_part[g] + inp_part[g] + b[g], then sigmoid/tanh gate
math and the prev_c recurrence.

Perf structure:
  - all matmuls bf16 (rel err ~5e-3, gate is 2e-2)
  - weights shipped in k-tile-major SBUF-image layout [128, ktiles*H1]
    so every weight DMA is long-contiguous per partition; streamed in
    slabs of up to 8 k-tiles ([2, 6] ramp for the level-1 weights)
  - ALL bulk DMAs pre-issued in PE need-order, alternating the two DGE
    queues (SP/Activation) pairwise: paired streams (v1/V1 vs v2/V2)
    pace each other and one slot-wait can't head-of-line block the rest
  - PSUM allocated in [128, 512] single-bank chunks, one tag rotating
    all 8 banks; hadamard consumes chunk-wise (bounce eats the stream
    that stops first, freeing the next stage's banks early)
  - Hadamard transposes moved off the PE onto the DMA XBAR
    (dma_start_transpose: out[p, t, b] = in[b, t*128+p], verified)
  - vis interleaves the V-chain and U-chain so the PE never waits on
    vector/XBAR between levels; last stages run m-outer so the first
    m-tile's output DMA overlaps the second's matmuls

The two programs are dispatched concurrently on disjoint device subsets
through a copy of concourse's PJRT runner that takes an explicit device
list (the stock one hardcodes jax.devices()[:n]).
"""

import numpy as np

import jax
from jax.experimental.shard_map import shard_map
from jax.sharding import Mesh, PartitionSpec

import concourse.mybir as mybir
import concourse.tile as tile
from concourse import bacc, bass2jax

B, X, V, MM, VH, H1, H2, G = 256, 12000, 4096, 1024, 1024, 1024, 1024, 4
XP = 12032  # X padded to a multiple of 128 (94 k-tiles)
N_CORES = 8
MT = 2      # m-tiles (batch 256 = 2 x 128)
NC2 = 2     # 512-col chunks per 1024 row (one PSUM bank each)

DT_NAME = "bfloat16"

_cache = {}


def _mm_dt():
    return getattr(mybir.dt, DT_NAME)


def _np_dt():
    return mybir.dt.np(_mm_dt())


def _slab_sizes(kt, ramp=False):
    """Slab progression. ramp=True starts [2, 6] so the PE begins after a
    512KB transfer; later-stage weights use full 8-k-tile slabs (bigger
    transfers = better per-queue DMA efficiency)."""
    sizes = []
    rem = kt
    if ramp:
        for s in (2, 6):
            s = min(s, rem)
            if s:
                sizes.append(s)
            rem -= s
    while rem > 0:
        sizes.append(min(8, rem))
        rem -= 8
    return sizes


def build_program(role):
    """role "vis": visual+recurrent paths; "inp": input path. Full batch."""
    dt = _mm_dt()
    f32 = mybir.dt.float32

    nc = bacc.Bacc("TRN2", target_bir_lowering=False, debug=False)

    if role == "vis":
        act_specs = {"v1T": V, "v2T": V, "mT": MM, "hT": H2}
        w_specs = {"V1": V, "V2": V, "C1": VH, "C2": MM, "C3": H1,
                   "U1": H2, "U2": MM, "U3": H1}
    else:
        act_specs = {"xT": XP, "mT": MM}
        w_specs = {"W1": XP, "W2": MM, "W3": H1}

    acts_d = {
        name: nc.dram_tensor(name, [128, k // 128 * B], dt, kind="ExternalInput")
        for name, k in act_specs.items()
    }
    # weights in k-tile-major SBUF-image layout [128, ktiles*H1]
    wt = {
        name: nc.dram_tensor(name, [128, (k // 128) * H1], dt, kind="ExternalInput")
        for name, k in w_specs.items()
    }
    out = nc.dram_tensor("out", [B, H2], f32, kind="ExternalOutput")

    with tile.TileContext(nc) as tc:
        with (
            tc.tile_pool(name="acts", bufs=1) as acts,
            # vis has SBUF headroom for deeper weight prefetch (inp's xT
            # image is 47KB/partition, so inp stays at 7)
            tc.tile_pool(name="wstream",
                         bufs=8 if role == "vis" else 7) as wstream,
            tc.tile_pool(name="inter", bufs=1) as inter,
            tc.tile_pool(name="ps", bufs=8, space="PSUM") as ps,
        ):
            # --- pre-plan ALL bulk DMAs in PE need-order, alternating the
            # two DGE queues pairwise so one slot-wait can't head-of-line
            # block the whole stream and paired streams pace together ---
            act_tiles = {}
            for name in act_specs:
                ktiles = act_specs[name] // 128
                t = acts.tile([128, ktiles * B], dt, tag=name, name=name)
                act_tiles[name] = t

            ramp_w = {"V1", "V2"} if role == "vis" else {"W1", "W2"}
            slab_plan = {}  # wname -> list of (k0, s, tile)
            for name in w_specs:
                plan, k0 = [], 0
                for s in _slab_sizes(w_specs[name] // 128, ramp=name in ramp_w):
                    w = wstream.tile([128, 8 * H1], dt, tag="w", name="w")
                    plan.append((k0, s, w))
                    k0 += s
                slab_plan[name] = plan

            def _sched(role_order):
                """role_order: ("act", name, (kt0, kt1)) / ("w", name, si).
                emit DMAs alternating queues in this exact order."""
                qi = 0
                for item in role_order:
                    eng = nc.sync if qi % 2 == 0 else nc.scalar
                    qi += 1
                    if item[0] == "act":
                        _, name, (t0, t1) = item
                        t1 = min(t1, act_specs[name] // 128)
                        if t0 >= t1:
                            continue
                        lo, hi = t0 * B, t1 * B
                        eng.dma_start(
                            act_tiles[name][:, lo:hi], acts_d[name].ap()[:, lo:hi]
                        )
                    else:
                        _, name, si = item
                        k0, s, w = slab_plan[name][si]
                        eng.dma_start(
                            w[:, :s * H1],
                            wt[name].ap()[:, k0 * H1:(k0 + s) * H1],
                        )

            act_sb = {
                name: act_tiles[name].rearrange("p (t m b) -> p t m b", m=MT, b=128)
                for name in act_specs
            }

            def alloc_ps(tag_name, nchunks=NC2):
                """[m][c] grid of single-bank [128,512] psum tiles."""
                return [
                    [ps.tile([128, 512], f32, tag="bank",
                             name=f"{tag_name}_{mi}_{c}") for c in range(nchunks)]
                    for mi in range(MT)
                ]

            def stream(jobs, m_outer=False):
                """jobs: list of (psums[m][c], act_fn(k, mi)->lhsT, wname).
                Matmuls only; the slab DMAs were pre-issued in need-order.
                Round-robin across jobs so paired streams finish together.
                m_outer: single job, loop m-tiles outermost so m0's psum
                accumulation completes mid-stage and its consumers overlap
                the m1 half."""
                if m_outer:
                    (psums, act, wname), = jobs
                    ktiles = w_specs[wname] // 128
                    for mi in range(MT):
                        for k0, s, w in slab_plan[wname]:
                            for dk in range(s):
                                k = k0 + dk
                                for c in range(NC2):
                                    nc.tensor.matmul(
                                        psums[mi][c][:],
                                        act(k, mi),
                                        w[:, dk * H1 + c * 512:
                                          dk * H1 + c * 512 + 512],
                                        start=k == 0,
                                        stop=k == ktiles - 1,
                                    )
                    return
                plans = []
                for psums, act, wname in jobs:
                    plans.append({
                        "psums": psums, "act": act, "wname": wname,
                        "ktiles": w_specs[wname] // 128,
                        "si": 0,
                    })
                while any(p["si"] < len(slab_plan[p["wname"]]) for p in plans):
                    for p in plans:
                        slabs = slab_plan[p["wname"]]
                        if p["si"] >= len(slabs):
                            continue
                        k0, s, w = slabs[p["si"]]
                        for dk in range(s):
                            k = k0 + dk
                            first = k == 0
                            last = k == p["ktiles"] - 1
                            for mi in range(MT):
                                lhsT = p["act"](k, mi)
                                for c in range(NC2):
                                    nc.tensor.matmul(
                                        p["psums"][mi][c][:],
                                        lhsT,
                                        w[:, dk * H1 + c * 512:
                                          dk * H1 + c * 512 + 512],
                                        start=first,
                                        stop=last,
                                    )
                        p["si"] += 1

            def hadamard(early, late, tag, bufs):
                """qT[m][128, t, 128] (bf16 SBUF) = transpose(early*late).
                `early` is the psum pair whose accumulation stops first: it
                is consumed by the bounce copies (so its banks — which the
                next stage reuses — free before the late stream even ends);
                `late` is consumed by the muls."""
                qTs = []
                for mi in range(MT):
                    q = inter.tile([128, H1], dt, tag="q", bufs=2, name="q")
                    for c in range(NC2):
                        bounce = inter.tile([128, 512], f32, tag="bounce",
                                            bufs=2, name="bounce")
                        nc.vector.tensor_copy(bounce[:], early[mi][c][:])
                        nc.vector.tensor_mul(
                            q[:, c * 512:(c + 1) * 512], late[mi][c][:], bounce[:]
                        )
                    qT = inter.tile([128, (H1 // 128) * 128], dt, tag=tag,
                                    bufs=bufs, name="qT")
                    qTv = qT.rearrange("p (t b) -> p t b", b=128)
                    nc.scalar.dma_start_transpose(qTv, q[:])
                    qTs.append(qTv)
                return qTs

            out_v = out.ap().rearrange("(m p) n -> m p n", p=128)

            def finish(l3_list):
                """acc[m] = sum of l3 psums (chunk-wise); one DMA per m-tile
                to keep the completion-semaphore chain short."""
                for mi in range(MT):
                    acc = inter.tile([128, H2], f32, tag="acc", bufs=2,
                                     name="acc")
                    for c in range(NC2):
                        sl = acc[:, c * 512:(c + 1) * 512]
                        nc.vector.tensor_copy(sl, l3_list[0][mi][c][:])
                        for l3 in l3_list[1:]:
                            nc.vector.tensor_add(sl, sl, l3[mi][c][:])
                    nc.sync.dma_start(out_v[mi], acc[:])

            if role == "vis":
                # DMA need-order: v-act chunks pace the V1/V2 slab streams
                # (V1+v1T on queue 0, V2+v2T on queue 1), then the small
                # later-stage weights split pairwise across both queues.
                order = []
                for i, (k0, s, _) in enumerate(slab_plan["V1"]):
                    order += [
                        ("act", "v1T", (k0, k0 + s)),
                        ("act", "v2T", (k0, k0 + s)),
                        ("w", "V1", i), ("w", "V2", i),
                    ]
                order += [("act", "mT", (0, 8)), ("act", "hT", (0, 8))]
                for nm in ("U1", "U2", "C1", "C2", "U3", "C3"):
                    for i in range(len(slab_plan[nm])):
                        order.append(("w", nm, i))
                _sched(order)

                # stage A: V level-1
                pa = alloc_ps("pa")
                pb = alloc_ps("pb")
                stream([
                    (pa, lambda k, mi: act_sb["v1T"][:, k, mi, :], "V1"),
                    (pb, lambda k, mi: act_sb["v2T"][:, k, mi, :], "V2"),
                ])
                t1T = hadamard(pb, pa, tag="qT", bufs=3)
                # stage B: U level-1 (fills PE while V hadamard runs)
                pau = alloc_ps("pau")
                pbu = alloc_ps("pbu")
                stream([
                    (pau, lambda k, mi: act_sb["hT"][:, k, mi, :], "U1"),
                    (pbu, lambda k, mi: act_sb["mT"][:, k, mi, :], "U2"),
                ])
                uT = hadamard(pbu, pau, tag="qT", bufs=3)
                # stage C: V level-2
                pa2 = alloc_ps("pa2")
                pb2 = alloc_ps("pb2")
                stream([
                    (pa2, lambda k, mi: t1T[mi][:, k, :], "C1"),
                    (pb2, lambda k, mi: act_sb["mT"][:, k, mi, :], "C2"),
                ])
                q2T = hadamard(pb2, pa2, tag="qT", bufs=3)
                # stage D: U level-3 (fills PE while V hadamard-2 runs)
                l3u = alloc_ps("l3u")
                stream([(l3u, lambda k, mi: uT[mi][:, k, :], "U3")],
                       m_outer=True)
                # stage E: V level-3
                l3v = alloc_ps("l3v")
                stream([(l3v, lambda k, mi: q2T[mi][:, k, :], "C3")],
                       m_outer=True)
                finish([l3u, l3v])
            else:
                # DMA need-order: ramp W2+W1 together, then W1 slab pairs
                # split across both queues, paced by their xT act chunks.
                nW2 = len(slab_plan["W2"])
                nW1 = len(slab_plan["W1"])
                order = [("act", "mT", (0, 8)), ("act", "xT", (0, 2))]
                order += [("w", "W2", 0), ("w", "W1", 0)]
                order += [("act", "xT", (2, 8)), ("w", "W2", 1), ("w", "W1", 1)]
                i = nW2
                while i < nW1:
                    for jj in (i, i + 1):
                        if jj < nW1:
                            k0, s, _ = slab_plan["W1"][jj]
                            order.append(("act", "xT", (k0, k0 + s)))
                    for jj in (i, i + 1):
                        if jj < nW1:
                            order.append(("w", "W1", jj))
                    i += 2
                for si in range(len(slab_plan["W3"])):
                    order.append(("w", "W3", si))
                _sched(order)

                # W2 first (small) so its psums sit in banks 0-3 while the
                # long W1 stream fills banks 4-7
                pb = alloc_ps("pb")
                pa = alloc_ps("pa")
                stream([
                    (pb, lambda k, mi: act_sb["mT"][:, k, mi, :], "W2"),
                    (pa, lambda k, mi: act_sb["xT"][:, k, mi, :], "W1"),
                ])
                xT_q = hadamard(pb, pa, tag="qT", bufs=2)
                l3w = alloc_ps("l3w")
                stream([(l3w, lambda k, mi: xT_q[mi][:, k, :], "W3")],
                       m_outer=True)
                finish([l3w])

    nc.compile()
    return nc


def _make_runner(nc, devices):
    """Adapted from concourse.bass2jax.run_bass_via_pjrt: same lowering,
    but runs on an explicit device subset and returns unmaterialized jax
    arrays so two programs can be dispatched concurrently."""
    bass2jax.install_neuronx_cc_hook()

    assert nc.dbg_addr is None
    partition_name = (
        nc.partition_id_tensor.name if nc.partition_id_tensor else None
    )

    in_names, out_names, out_avals, zero_outs = [], [], [], []
    for alloc in nc.m.functions[0].allocations:
        if not isinstance(alloc, mybir.MemoryLocationSet):
            continue
        name = alloc.memorylocations[0].name
        if alloc.kind == "ExternalInput":
            if name != partition_name:
                in_names.append(name)
        elif alloc.kind == "ExternalOutput":
            shape = tuple(alloc.tensor_shape)
            dtype = mybir.dt.np(alloc.dtype)
            out_names.append(name)
            out_avals.append(jax.core.ShapedArray(shape, dtype))
            zero_outs.append(np.zeros(shape, dtype))
    n_params = len(in_names)
    n_outs = len(out_avals)
    in_names.extend(out_names)
    if partition_name is not None:
        in_names.append(partition_name)
    donate = tuple(range(n_params, n_params + n_outs))

    def _body(*args):
        operands = list(args)
        if partition_name is not None:
            operands.append(bass2jax.partition_id_tensor())
        outs = bass2jax._bass_exec_p.bind(
            *operands,
            out_avals=tuple(out_avals),
            in_names=tuple(in_names),
            out_names=tuple(out_names),
            lowering_input_output_aliases=(),
            sim_require_finite=True,
            sim_require_nnan=True,
            nc=nc,
        )
        return tuple(outs)

    n_cores = len(devices)
    mesh = Mesh(np.asarray(devices), ("core",))
    in_specs = (PartitionSpec("core"),) * (n_params + n_outs)
    out_specs = (PartitionSpec("core"),) * n_outs
    sharded = jax.jit(
        shard_map(
            _body, mesh=mesh, in_specs=in_specs, out_specs=out_specs,
            check_rep=False,
        ),
        donate_argnums=donate,
        keep_unused=True,
    )

    def run(in_maps):
        assert len(in_maps) == n_cores
        concat_in = [
            np.concatenate(
                [np.asarray(in_maps[c][name]) for c in range(n_cores)], axis=0
            )
            for name in in_names[:n_params]
        ]
        concat_zeros = [
            np.zeros((n_cores * z.shape[0], *z.shape[1:]), z.dtype)
            for z in zero_outs
        ]
        out_arrs = sharded(*concat_in, *concat_zeros)
        return out_names, out_avals, out_arrs

    return run


def _tile_actT(a, kdim):
    """[256 batch, K<=kdim] -> SBUF image [128, (kdim/128) * 256]:
    (p, (t*2+mi)*128+b) = a[mi*128+b, t*128+p], contiguous per partition."""
    ktiles = kdim // 128
    a = np.asarray(a, np.float32)
    if a.shape[1] < kdim:
        a = np.pad(a, ((0, 0), (0, kdim - a.shape[1])))
    # [2m, 128b, ktiles, 128p] -> [128p, ktiles, 2m, 128b]
    r = a.reshape(MT, 128, ktiles, 128).transpose(3, 2, 0, 1)
    return np.ascontiguousarray(r.reshape(128, ktiles * B), dtype=_np_dt())


def _w_img(w, kdim):
    """[K<=kdim, H1] -> k-tile-major SBUF image [128, (kdim/128)*H1]:
    (p, t*H1 + n) = w[t*128+p, n], long-contiguous per partition."""
    kt = kdim // 128
    a = np.asarray(w, np.float32)
    if a.shape[0] < kdim:
        a = np.pad(a, ((0, kdim - a.shape[0]), (0, 0)))
    img = a.reshape(kt, 128, H1).transpose(1, 0, 2).reshape(128, kt * H1)
    return np.ascontiguousarray(img, dtype=_np_dt())


def kernel(prev_h, prev_c, x, m, v1, v2, V1, V2, C1, C2, C3, W1, W2, W3, U1, U2, U3, b):
    if "runners" not in _cache:
        devs = jax.devices()
        nc_vis = build_program("vis")
        nc_inp = build_program("inp")
        _cache["runners"] = (
            _make_runner(nc_vis, devs[0:4]),
            _make_runner(nc_inp, devs[4:8]),
        )
        _cache["ncs"] = (nc_vis, nc_inp)
    run_vis, run_inp = _cache["runners"]

    v1T_img = _tile_actT(v1, V)
    v2T_img = _tile_actT(v2, V)
    mT_img = _tile_actT(m, MM)
    hT_img = _tile_actT(prev_h, H2)
    xT_img = _tile_actT(x, XP)

    vis_maps, inp_maps = [], []
    for g in range(G):
        vis_maps.append({
            "v1T": v1T_img, "v2T": v2T_img, "mT": mT_img, "hT": hT_img,
            "V1": _w_img(V1[g], V),
            "V2": _w_img(V2[g], V),
            "C1": _w_img(C1[g], VH),
            "C2": _w_img(C2[g], MM),
            "C3": _w_img(C3[g], H1),
            "U1": _w_img(U1[g], H2),
            "U2": _w_img(U2[g], MM),
            "U3": _w_img(U3[g], H1),
        })
        inp_maps.append({
            "xT": xT_img, "mT": mT_img,
            "W1": _w_img(W1[g], XP),
            "W2": _w_img(W2[g], MM),
            "W3": _w_img(W3[g], H1),
        })

    _cache["last_in_maps"] = (vis_maps, inp_maps)

    # dispatch both programs; they run concurrently on disjoint cores
    vnames, vavals, vouts = run_vis(vis_maps)
    inames, iavals, iouts = run_inp(inp_maps)

    vis_out = np.asarray(vouts[0]).reshape(G, B, H2)
    inp_out = np.asarray(iouts[0]).reshape(G, B, H2)

    logits = vis_out + inp_out + np.asarray(b, np.float32)[:, None, :]

    def sigmoid(z):
        return 1.0 / (1.0 + np.exp(-z))

    i = sigmoid(logits[0])
    f = sigmoid(logits[1])
    o = sigmoid(logits[2])
    cg = np.tanh(logits[3])
    prev_c = np.asarray(prev_c, np.float32)
    new_c = f * prev_c + i * cg
    new_h = o * np.tanh(prev_c)
    return new_h.astype(np.float32), new_c.astype(np.float32)
